# revision 19
# baseline (speedup 1.0000x reference)
"""Trainium2 Bass kernel for a pre-LN causal-attention transformer block.

Reference computation (fp32):
    h1 = LN(x; g1, b1)
    q,k,v = per-head projections of h1;  causal softmax attention
    x2 = x + (attn_out @ wp) + bp
    h2 = LN(x2; g2, b2)
    out = x2 + relu(h2 @ w1) @ w2

Sharding: data-parallel over batch. B=64 -> 8 batches per NeuronCore.
Each core runs the full block on its 8 batches; no collectives.

Per-core dataflow (tokens = 8*256 = 2048, processed in 4 chunks of 512):
  - LN stats in token-major layout (reductions over the free dim), with the
    LN affine (g,b) folded into the following weights on the host.
  - PE transposes produce feature-major activations for the stationary-
    weight matmuls (fp32r: full-rate with ~1e-4 rounding).
  - Attention internals (Q^T, K^T, V, probs) in bf16: scores via K=64
    head-pair row-packed matmuls, softmax via ACT Exp with fused row-sum
    accumulation (no max subtraction: scores are provably tiny), PE
    transposes of the prob tiles, and an fp32 PSUM accumulation for
    attn @ V with head-pair column packing.
"""
import numpy as np
import ml_dtypes

import concourse.tile as tile
from concourse import bacc, mybir
from concourse.bass_utils import run_bass_kernel_spmd

F32 = mybir.dt.float32
F32R = mybir.dt.float32r
BF16 = mybir.dt.bfloat16
AF = mybir.ActivationFunctionType
ALU = mybir.AluOpType

B, T, C = 64, 256, 384
H, HD = 6, 64
FF = 4 * C                      # 1536
NCORES = 8
BL = B // NCORES                # 8 batches per core
TOK = BL * T                    # 2048 tokens per core
CHB = 2                         # batches per chunk
NCH = BL // CHB                 # 4 chunks
CTOK = CHB * T                  # 512 tokens per chunk
NTT = CTOK // 128               # 4 token tiles per chunk
EPS = 1e-5
NEG = -30.0                     # additive causal mask value (exp(-30) ~ 1e-13)

# engine-assignment knobs (tuned via TimelineSim + HW)
CFG = dict(
    s1_engine="vector",     # LN sum(x): free-axis reduce is DVE-only
    qk_copy="scalar",       # QT/KT psum->sbuf+bias: "scalar" | "vector"
    ot_copy="scalar",       # oT copies: "scalar" | "vector"
    relu_split=2,           # 0=all ACT, 1=all DVE, n>=2: fm%n==1 -> DVE
    h_copy="scalar",        # h1T/h2T copies
    at_mode="pe",           # attn prob transposes: "dma" | "pe"
)

_CACHE = {}


def _build(debug=False, repeat=1):
    nc = bacc.Bacc("TRN2", target_bir_lowering=False, debug=False,
                   num_devices=NCORES)

    # ---- DRAM I/O ----------------------------------------------------------
    x_d = nc.dram_tensor("x", [TOK, C], F32, kind="ExternalInput").ap()
    wq_d = nc.dram_tensor("wq", [C, C], F32R, kind="ExternalInput").ap()
    wk_d = nc.dram_tensor("wk", [C, C], F32R, kind="ExternalInput").ap()
    wv_d = nc.dram_tensor("wv", [C, C], F32R, kind="ExternalInput").ap()
    wp_d = nc.dram_tensor("wp", [C, C], F32R, kind="ExternalInput").ap()
    w1_d = nc.dram_tensor("w1", [C, FF], F32R, kind="ExternalInput").ap()
    w2_d = nc.dram_tensor("w2", [FF, C], F32R, kind="ExternalInput").ap()
    cqk_d = nc.dram_tensor("cqk", [128, 6], F32, kind="ExternalInput").ap()
    c1_d = nc.dram_tensor("c1", [128, 12], F32, kind="ExternalInput").ap()
    cvb_d = nc.dram_tensor("cvb", [128, C], F32, kind="ExternalInput").ap()
    bpb_d = nc.dram_tensor("bpb", [128, C], F32, kind="ExternalInput").ap()
    idr_d = nc.dram_tensor("idr", [128, 128], F32R, kind="ExternalInput").ap()
    idb_d = nc.dram_tensor("idb", [128, 128], BF16, kind="ExternalInput").ap()
    cmask_d = nc.dram_tensor("cmask", [128, 3 * 128], F32, kind="ExternalInput").ap()
    out_d = nc.dram_tensor("out", [TOK, C], F32, kind="ExternalOutput").ap()
    dbg = {}
    if debug:
        for nm, shape, dt in [
            ("d_h1", [128, NTT * C], F32R), ("d_h1T", [128, 3 * CTOK], F32R),
            ("d_qT", [128, 3 * CTOK], BF16), ("d_kT", [128, 3 * CTOK], BF16),
            ("d_v", [128, NTT * C], BF16), ("d_a0", [128, 128], BF16),
            ("d_a1", [128, 256], BF16), ("d_aT0", [128, 256], BF16),
            ("d_oT", [128, 3 * CTOK], F32R), ("d_x2", [128, NTT * C], F32),
            ("d_a1T", [128, 12 * CTOK], F32R), ("d_sums", [128, 2 * H], F32),
        ]:
            dbg[nm] = nc.dram_tensor(nm, shape, dt, kind="ExternalOutput").ap()

    with tile.TileContext(nc) as tc:
        with (
            tc.tile_pool(name="const", bufs=1) as cp,
            tc.tile_pool(name="io", bufs=2) as iop,
            tc.tile_pool(name="act", bufs=1) as ap_,
            tc.tile_pool(name="attn", bufs=13) as atp,
            tc.tile_pool(name="attnT", bufs=3) as aTp,
            tc.tile_pool(name="small", bufs=4) as smp,
            tc.tile_pool(name="mmps", bufs=2, space="PSUM") as mmp,
            tc.tile_pool(name="scps", bufs=3, space="PSUM") as scp,
            tc.tile_pool(name="tpps", bufs=2, space="PSUM") as tpp,
            tc.tile_pool(name="ops", bufs=1, space="PSUM") as opp,
        ):
            # ---- persistent weights / constants (issue order = first use) --
            idr_s = cp.tile([128, 128], F32R)
            nc.sync.dma_start(idr_s[:], idr_d[:])
            x_first = iop.tile([128, NTT * C], F32, tag="x")
            nc.sync.dma_start(
                x_first[:].rearrange("p (t c) -> p t c", t=NTT),
                x_d[0:CTOK, :].rearrange("(t p) c -> p t c", p=128))
            wq_s = cp.tile([128, 3 * C], F32R)
            wk_s = cp.tile([128, 3 * C], F32R)
            for c in range(3):
                nc.sync.dma_start(wq_s[:, C * c:C * (c + 1)], wq_d[128 * c:128 * (c + 1), :])
                nc.sync.dma_start(wk_s[:, C * c:C * (c + 1)], wk_d[128 * c:128 * (c + 1), :])
            cqk_s = cp.tile([128, 6], F32)
            nc.sync.dma_start(cqk_s[:], cqk_d[:])
            wv_s = cp.tile([128, 3 * C], F32R)
            for c in range(3):
                nc.sync.dma_start(wv_s[:, C * c:C * (c + 1)], wv_d[128 * c:128 * (c + 1), :])
            cvb_s = cp.tile([128, C], F32)
            nc.sync.dma_start(cvb_s[:], cvb_d[:])
            bpb_s = cp.tile([128, C], F32)
            nc.sync.dma_start(bpb_s[:], bpb_d[:])
            cmask_s = cp.tile([128, 3 * 128], F32)
            nc.sync.dma_start(cmask_s[:], cmask_d[:])
            idb_s = cp.tile([128, 128], BF16)
            nc.sync.dma_start(idb_s[:], idb_d[:])
            wp_s = cp.tile([128, 3 * C], F32R)
            for c in range(3):
                nc.sync.dma_start(wp_s[:, C * c:C * (c + 1)], wp_d[128 * c:128 * (c + 1), :])
            w1_s = cp.tile([128, 3 * FF], F32R)
            for c in range(3):
                nc.sync.dma_start(w1_s[:, FF * c:FF * (c + 1)], w1_d[128 * c:128 * (c + 1), :])
            c1_s = cp.tile([128, 12], F32)
            nc.sync.dma_start(c1_s[:], c1_d[:])
            w2_s = cp.tile([128, 12 * C], F32R)
            for f in range(12):
                nc.sync.dma_start(w2_s[:, C * f:C * (f + 1)], w2_d[128 * f:128 * (f + 1), :])

            def load_x(ch):
                base = (ch % NCH) * CTOK
                t = iop.tile([128, NTT * C], F32, tag="x")
                nc.sync.dma_start(
                    t[:].rearrange("p (t c) -> p t c", t=NTT),
                    x_d[base:base + CTOK, :].rearrange("(t p) c -> p t c", p=128))
                return t

            x_next = x_first
            for ch in range(NCH * repeat):
                ch_next = ch + 1
                ch = ch % NCH
                base = ch * CTOK

                x_sb = x_next
                if ch_next < NCH * repeat:
                    x_next = load_x(ch_next)

                # ---- LN1 (token-major) -> h1_z (fp32r) ---------------------
                s1 = smp.tile([128, NTT], F32, tag="s1")
                s2 = smp.tile([128, NTT], F32, tag="s2")
                sq = ap_.tile([128, C], F32, tag="sq")
                for tt in range(NTT):
                    xt = x_sb[:, C * tt:C * (tt + 1)]
                    getattr(nc, CFG["s1_engine"]).reduce_sum(
                        s1[:, tt:tt + 1], xt, axis=mybir.AxisListType.X)
                    nc.scalar.activation(sq[:], xt, AF.Square, bias=0.0, scale=1.0,
                                         accum_out=s2[:, tt:tt + 1])
                mu = smp.tile([128, NTT], F32, tag="mu")
                nc.vector.tensor_scalar_mul(mu[:], s1[:], 1.0 / C)
                var = smp.tile([128, NTT], F32, tag="var")
                nc.vector.tensor_scalar(var[:], s2[:], 1.0 / C, EPS,
                                        op0=ALU.mult, op1=ALU.add)
                musq = smp.tile([128, NTT], F32, tag="musq")
                nc.vector.tensor_tensor(musq[:], mu[:], mu[:], op=ALU.mult)
                nc.vector.tensor_tensor(var[:], var[:], musq[:], op=ALU.subtract)
                sd = smp.tile([128, NTT], F32, tag="sd")
                nc.scalar.activation(sd[:], var[:], AF.Sqrt, bias=0.0, scale=1.0)
                rstd = smp.tile([128, NTT], F32, tag="rstd")
                nc.vector.reciprocal(rstd[:], sd[:])

                h1 = ap_.tile([128, NTT * C], F32R, tag="h1")
                for tt in range(NTT):
                    nc.vector.tensor_scalar(
                        h1[:, C * tt:C * (tt + 1)], x_sb[:, C * tt:C * (tt + 1)],
                        mu[:, tt:tt + 1], rstd[:, tt:tt + 1],
                        op0=ALU.subtract, op1=ALU.mult)

                # ---- transpose h1 -> h1T [c-tile partitions, tokens] -------
                h1T = ap_.tile([128, 3 * CTOK], F32R, tag="h1T")
                for c in range(3):
                    tp = scp.tile([128, CTOK], F32R, tag="sc")
                    for tt in range(NTT):
                        nc.tensor.matmul(
                            tp[:, 128 * tt:128 * (tt + 1)],
                            h1[:, C * tt + 128 * c:C * tt + 128 * (c + 1)], idr_s[:],
                            is_transpose=True, start=(tt == 0), stop=(tt == NTT - 1))
                    if CFG["h_copy"] == "scalar":
                        nc.scalar.copy(h1T[:, CTOK * c:CTOK * (c + 1)], tp[:])
                    else:
                        nc.vector.tensor_copy(h1T[:, CTOK * c:CTOK * (c + 1)], tp[:])

                if debug and ch == 0:
                    nc.sync.dma_start(dbg["d_h1"][:], h1[:])
                    nc.sync.dma_start(dbg["d_h1T"][:], h1T[:])

                # ---- Q^T, K^T (feature-major, bf16, bias folded) -----------
                qT = ap_.tile([128, 3 * CTOK], BF16, tag="qT")
                kT = ap_.tile([128, 3 * CTOK], BF16, tag="kT")
                for w_s, oT, bcol in ((wq_s, qT, 0), (wk_s, kT, 3)):
                    for m in range(3):
                        ps = mmp.tile([128, CTOK], F32, tag="mm")
                        for c in range(3):
                            nc.tensor.matmul(
                                ps[:],
                                w_s[:, C * c + 128 * m:C * c + 128 * (m + 1)],
                                h1T[:, CTOK * c:CTOK * (c + 1)],
                                start=(c == 0), stop=(c == 2))
                        if CFG["qk_copy"] == "scalar":
                            nc.scalar.activation(
                                oT[:, CTOK * m:CTOK * (m + 1)], ps[:], AF.Identity,
                                bias=cqk_s[:, bcol + m:bcol + m + 1], scale=1.0)
                        else:
                            nc.vector.tensor_scalar_add(
                                oT[:, CTOK * m:CTOK * (m + 1)], ps[:],
                                cqk_s[:, bcol + m:bcol + m + 1])

                # ---- V (token-major, bf16, bias folded) --------------------
                v_sb = ap_.tile([128, NTT * C], BF16, tag="v")
                for tt in range(NTT):
                    ps = mmp.tile([128, C], F32, tag="mm")
                    for c in range(3):
                        nc.tensor.matmul(
                            ps[:],
                            h1T[:, CTOK * c + 128 * tt:CTOK * c + 128 * (tt + 1)],
                            wv_s[:, C * c:C * (c + 1)],
                            start=(c == 0), stop=(c == 2))
                    nc.vector.tensor_tensor(
                        v_sb[:, C * tt:C * (tt + 1)], ps[:], cvb_s[:], op=ALU.add)

                if debug and ch == 0:
                    nc.sync.dma_start(dbg["d_qT"][:], qT[:])
                    nc.sync.dma_start(dbg["d_kT"][:], kT[:])
                    nc.sync.dma_start(dbg["d_v"][:], v_sb[:])

                # ---- attention, per batch in chunk -------------------------
                oT = ap_.tile([128, 3 * CTOK], F32R, tag="oT")
                x2 = ap_.tile([128, NTT * C], F32, tag="x2")
                for bb in range(CHB):
                    t0, t1 = 2 * bb, 2 * bb + 1     # token tiles of this batch
                    sums = smp.tile([128, 2 * H], F32, tag="sums")
                    attn0 = []
                    attn1 = []
                    for h in range(H):
                        qp = 64 * (h % 2)
                        qm = h // 2
                        q_t0 = qT[qp:qp + 64, CTOK * qm + 128 * t0:CTOK * qm + 128 * (t0 + 1)]
                        q_t1 = qT[qp:qp + 64, CTOK * qm + 128 * t1:CTOK * qm + 128 * (t1 + 1)]
                        k_s0 = kT[qp:qp + 64, CTOK * qm + 128 * t0:CTOK * qm + 128 * (t0 + 1)]
                        k_s01 = kT[qp:qp + 64, CTOK * qm + 128 * t0:CTOK * qm + 128 * (t0 + 2)]

                        sc = scp.tile([128, 384], F32, tag="sc")
                        nc.tensor.matmul(sc[:, 0:128], q_t0, k_s0, start=True, stop=False)
                        nc.tensor.matmul(sc[:, 128:384], q_t1, k_s01, start=False, stop=True)
                        nc.vector.tensor_tensor(sc[:], sc[:], cmask_s[:], op=ALU.add)
                        a0 = atp.tile([128, 128], BF16, tag="a0")
                        nc.scalar.activation(a0[:], sc[:, 0:128], AF.Exp, bias=0.0, scale=1.0,
                                             accum_out=sums[:, 2 * h:2 * h + 1])
                        attn0.append(a0)
                        a1 = atp.tile([128, 256], BF16, tag="a1")
                        nc.scalar.activation(a1[:], sc[:, 128:384], AF.Exp, bias=0.0, scale=1.0,
                                             accum_out=sums[:, 2 * h + 1:2 * h + 2])
                        attn1.append(a1)

                    rec = smp.tile([128, 2 * H], F32, tag="rec")
                    nc.vector.reciprocal(rec[:], sums[:])

                    for h in range(H):
                        a0, a1 = attn0[h], attn1[h]
                        nc.vector.tensor_scalar_mul(a0[:], a0[:], rec[:, 2 * h:2 * h + 1])
                        nc.vector.tensor_scalar_mul(a1[:], a1[:], rec[:, 2 * h + 1:2 * h + 2])

                        # transpose probs:  aT_s0 [s0, 256 t], aT_s1 [s1, t1 only]
                        aT0 = aTp.tile([128, 256], BF16, tag="aT0")
                        aT1 = aTp.tile([128, 128], BF16, tag="aT1")
                        if CFG["at_mode"] == "dma":
                            nc.sync.dma_start_transpose(aT0[:, 0:128], a0[:])
                            nc.sync.dma_start_transpose(aT0[:, 128:256], a1[:, 0:128])
                            nc.sync.dma_start_transpose(aT1[:], a1[:, 128:256])
                        else:
                            tp0 = tpp.tile([128, 256], BF16, tag="tpa")
                            nc.tensor.matmul(tp0[:, 0:128], a0[:], idb_s[:],
                                             is_transpose=True, start=True, stop=False)
                            nc.tensor.matmul(tp0[:, 128:256], a1[:, 0:128], idb_s[:],
                                             is_transpose=True, start=False, stop=True)
                            nc.vector.tensor_copy(aT0[:], tp0[:])
                            tp2 = tpp.tile([128, 256], BF16, tag="tpa")
                            nc.tensor.matmul(tp2[:, 0:128], a1[:, 128:256], idb_s[:],
                                             is_transpose=True, start=True, stop=True)
                            nc.vector.tensor_copy(aT1[:], tp2[:, 0:128])

                        if debug and ch == 0 and bb == 0 and h == 0:
                            nc.sync.dma_start(dbg["d_a0"][:], a0[:])
                            nc.sync.dma_start(dbg["d_a1"][:], a1[:])
                            nc.sync.dma_start(dbg["d_aT0"][:], aT0[:])

                        # attn @ V -> OT psum [64 d, 256 t] per head
                        ot_ps = opp.tile([64, 256], F32, tag="ot")
                        nc.tensor.matmul(
                            ot_ps[:],
                            v_sb[:, C * t0 + 64 * h:C * t0 + 64 * (h + 1)],
                            aT0[:], start=True, stop=False)
                        nc.tensor.matmul(
                            ot_ps[:, 128:256],
                            v_sb[:, C * t1 + 64 * h:C * t1 + 64 * (h + 1)],
                            aT1[:], start=False, stop=True)
                        hp, op = h // 2, 64 * (h % 2)
                        dst = oT[op:op + 64, CTOK * hp + 256 * bb:CTOK * hp + 256 * (bb + 1)]
                        if CFG["ot_copy"] == "scalar":
                            nc.scalar.copy(dst, ot_ps[:])
                        else:
                            nc.vector.tensor_copy(dst, ot_ps[:])

                    # ---- proj + residual + bp for this batch's tiles -------
                    for tt in (t0, t1):
                        pps = mmp.tile([128, C], F32, tag="mm")
                        for c in range(3):
                            nc.tensor.matmul(
                                pps[:],
                                oT[:, CTOK * c + 128 * tt:CTOK * c + 128 * (tt + 1)],
                                wp_s[:, C * c:C * (c + 1)],
                                start=(c == 0), stop=(c == 2))
                        nc.vector.tensor_tensor(
                            pps[:], pps[:], x_sb[:, C * tt:C * (tt + 1)], op=ALU.add)
                        nc.vector.tensor_tensor(
                            x2[:, C * tt:C * (tt + 1)], pps[:], bpb_s[:], op=ALU.add)

                if debug and ch == 0:
                    nc.sync.dma_start(dbg["d_oT"][:], oT[:])
                    nc.sync.dma_start(dbg["d_sums"][:], sums[:])


                if debug and ch == 0:
                    nc.sync.dma_start(dbg["d_x2"][:], x2[:])

                # ---- LN2 -> h2_z (fp32r) -----------------------------------
                for tt in range(NTT):
                    xt = x2[:, C * tt:C * (tt + 1)]
                    getattr(nc, CFG["s1_engine"]).reduce_sum(
                        s1[:, tt:tt + 1], xt, axis=mybir.AxisListType.X)
                    nc.scalar.activation(sq[:], xt, AF.Square, bias=0.0, scale=1.0,
                                         accum_out=s2[:, tt:tt + 1])
                nc.vector.tensor_scalar_mul(mu[:], s1[:], 1.0 / C)
                nc.vector.tensor_scalar(var[:], s2[:], 1.0 / C, EPS,
                                        op0=ALU.mult, op1=ALU.add)
                nc.vector.tensor_tensor(musq[:], mu[:], mu[:], op=ALU.mult)
                nc.vector.tensor_tensor(var[:], var[:], musq[:], op=ALU.subtract)
                nc.scalar.activation(sd[:], var[:], AF.Sqrt, bias=0.0, scale=1.0)
                nc.vector.reciprocal(rstd[:], sd[:])
                h2 = ap_.tile([128, NTT * C], F32R, tag="h2")
                for tt in range(NTT):
                    nc.vector.tensor_scalar(
                        h2[:, C * tt:C * (tt + 1)], x2[:, C * tt:C * (tt + 1)],
                        mu[:, tt:tt + 1], rstd[:, tt:tt + 1],
                        op0=ALU.subtract, op1=ALU.mult)

                h2T = ap_.tile([128, 3 * CTOK], F32R, tag="h2T")
                for c in range(3):
                    tp = scp.tile([128, CTOK], F32R, tag="sc")
                    for tt in range(NTT):
                        nc.tensor.matmul(
                            tp[:, 128 * tt:128 * (tt + 1)],
                            h2[:, C * tt + 128 * c:C * tt + 128 * (c + 1)], idr_s[:],
                            is_transpose=True, start=(tt == 0), stop=(tt == NTT - 1))
                    if CFG["h_copy"] == "scalar":
                        nc.scalar.copy(h2T[:, CTOK * c:CTOK * (c + 1)], tp[:])
                    else:
                        nc.vector.tensor_copy(h2T[:, CTOK * c:CTOK * (c + 1)], tp[:])

                # ---- MLP1: a1T = relu(w1'^T h2T + c1)  (fp32r) -------------
                a1T = ap_.tile([128, 12 * CTOK], F32R, tag="a1T")
                for fm in range(12):
                    ps = mmp.tile([128, CTOK], F32, tag="mm")
                    for c in range(3):
                        nc.tensor.matmul(
                            ps[:],
                            w1_s[:, FF * c + 128 * fm:FF * c + 128 * (fm + 1)],
                            h2T[:, CTOK * c:CTOK * (c + 1)],
                            start=(c == 0), stop=(c == 2))
                    if CFG["relu_split"] == 1 or (
                            CFG["relu_split"] > 1 and fm % CFG["relu_split"] == 1):
                        nc.vector.tensor_scalar(
                            a1T[:, CTOK * fm:CTOK * (fm + 1)], ps[:],
                            c1_s[:, fm:fm + 1], 0.0, op0=ALU.add, op1=ALU.max)
                    else:
                        nc.scalar.activation(
                            a1T[:, CTOK * fm:CTOK * (fm + 1)], ps[:], AF.Relu,
                            bias=c1_s[:, fm:fm + 1], scale=1.0)

                if debug and ch == 0:
                    nc.sync.dma_start(dbg["d_a1T"][:], a1T[:])

                # ---- MLP2 + residual -> out --------------------------------
                o_sb = iop.tile([128, NTT * C], F32, tag="o")
                for tt in range(NTT):
                    ps = mmp.tile([128, C], F32, tag="mm")
                    for fm in range(12):
                        nc.tensor.matmul(
                            ps[:],
                            a1T[:, CTOK * fm + 128 * tt:CTOK * fm + 128 * (tt + 1)],
                            w2_s[:, C * fm:C * (fm + 1)],
                            start=(fm == 0), stop=(fm == 11))
                    nc.vector.tensor_tensor(
                        o_sb[:, C * tt:C * (tt + 1)], ps[:],
                        x2[:, C * tt:C * (tt + 1)], op=ALU.add)
                nc.sync.dma_start(
                    out_d[base:base + CTOK, :].rearrange("(t p) c -> p t c", p=128),
                    o_sb[:].rearrange("p (t c) -> p t c", t=NTT))

    nc.compile()
    return nc


def _prep_inputs(x, wq, wk, wv, wp, bp, w1, w2, g1, b1, g2, b2):
    """Host-side weight folding + per-core input maps."""
    f32 = np.float32
    scale = HD ** -0.5
    wq_m = np.ascontiguousarray(wq.transpose(1, 0, 2).reshape(C, C))
    wk_m = np.ascontiguousarray(wk.transpose(1, 0, 2).reshape(C, C))
    wv_m = np.ascontiguousarray(wv.transpose(1, 0, 2).reshape(C, C))
    wq_p = (g1[:, None] * wq_m * scale).astype(f32)
    cq = (b1 @ wq_m * scale).astype(f32)
    wk_p = (g1[:, None] * wk_m).astype(f32)
    ck = (b1 @ wk_m).astype(f32)
    wv_p = (g1[:, None] * wv_m).astype(f32)
    cv = (b1 @ wv_m).astype(f32)
    w1_p = (g2[:, None] * w1).astype(f32)
    c1 = (b2 @ w1).astype(f32)

    cqk = np.concatenate([cq.reshape(3, 128).T, ck.reshape(3, 128).T], axis=1)
    cqk = np.ascontiguousarray(cqk, dtype=f32)                       # [128, 6]
    c1t = np.ascontiguousarray(c1.reshape(12, 128).T, dtype=f32)     # [128, 12]
    cvb = np.ascontiguousarray(np.broadcast_to(cv, (128, C)), dtype=f32)
    bpb = np.ascontiguousarray(np.broadcast_to(np.asarray(bp, f32), (128, C)))
    idr = np.eye(128, dtype=f32)
    idb = np.eye(128).astype(ml_dtypes.bfloat16)
    ii, jj = np.arange(128)[:, None], np.arange(128)[None, :]
    tri = np.where(jj <= ii, 0.0, NEG).astype(f32)
    cmask = np.concatenate([tri, np.zeros((128, 128), f32), tri], axis=1)

    common = dict(wq=wq_p, wk=wk_p, wv=wv_p, wp=np.asarray(wp, f32),
                  w1=w1_p, w2=np.asarray(w2, f32), cqk=cqk, c1=c1t,
                  cvb=cvb, bpb=bpb, idr=idr, idb=idb, cmask=cmask)
    in_maps = []
    for core in range(NCORES):
        xs = np.ascontiguousarray(
            x[BL * core:BL * (core + 1)].reshape(TOK, C), dtype=f32)
        in_maps.append(dict(common, x=xs))
    return in_maps


def run(inputs, trace=False, trace_kwargs=None, debug=False):
    """Compile (cached), run on 8 cores, gather. Returns (out, results)."""
    key = "nc_dbg" if debug else "nc"
    if key not in _CACHE:
        _CACHE[key] = _build(debug=debug)
    nc = _CACHE[key]
    in_maps = _prep_inputs(**inputs)
    res = run_bass_kernel_spmd(nc, in_maps, list(range(NCORES)),
                               trace=trace, **(trace_kwargs or {}))
    out = np.empty((B, T, C), dtype=np.float32)
    for core in range(NCORES):
        out[BL * core:BL * (core + 1)] = \
            res.results[core]["out"].reshape(BL, T, C)
    return out, res


def kernel(**inputs):
    out, _ = run(inputs)
    return out


# revision 22
# speedup vs baseline: 1.0733x; 1.0733x over previous
"""Trainium2 Bass kernel for a pre-LN causal-attention transformer block.

Reference computation (fp32):
    h1 = LN(x; g1, b1)
    q,k,v = per-head projections of h1;  causal softmax attention
    x2 = x + (attn_out @ wp) + bp
    h2 = LN(x2; g2, b2)
    out = x2 + relu(h2 @ w1) @ w2

Sharding: data-parallel over batch. B=64 -> 8 batches per NeuronCore.
Each core runs the full block on its 8 batches; no collectives.

Per-core dataflow (tokens = 8*256 = 2048, processed in 4 chunks of 512):
  - LN stats in token-major layout (reductions over the free dim), with the
    LN affine (g,b) folded into the following weights on the host.
  - PE transposes produce feature-major activations for the stationary-
    weight matmuls (fp32r: full-rate with ~1e-4 rounding).
  - Attention internals (Q^T, K^T, V, probs) in bf16: scores via K=64
    head-pair row-packed matmuls, softmax via ACT Exp with fused row-sum
    accumulation (no max subtraction: scores are provably tiny), PE
    transposes of the prob tiles, and an fp32 PSUM accumulation for
    attn @ V with head-pair column packing.
"""
import numpy as np
import ml_dtypes

import concourse.tile as tile
from concourse import bacc, mybir
from concourse.bass_utils import run_bass_kernel_spmd

F32 = mybir.dt.float32
F32R = mybir.dt.float32r
BF16 = mybir.dt.bfloat16
AF = mybir.ActivationFunctionType
ALU = mybir.AluOpType

B, T, C = 64, 256, 384
H, HD = 6, 64
FF = 4 * C                      # 1536
NCORES = 8
BL = B // NCORES                # 8 batches per core
TOK = BL * T                    # 2048 tokens per core
CHB = 2                         # batches per chunk
NCH = BL // CHB                 # 4 chunks
CTOK = CHB * T                  # 512 tokens per chunk
NTT = CTOK // 128               # 4 token tiles per chunk
EPS = 1e-5
NEG = -30.0                     # additive causal mask value (exp(-30) ~ 1e-13)

# engine-assignment knobs (tuned via TimelineSim + HW)
CFG = dict(
    s1_engine="vector",     # LN sum(x): free-axis reduce is DVE-only
    qk_copy="scalar",       # QT/KT psum->sbuf+bias: "scalar" | "vector"
    ot_copy="scalar",       # oT copies: "scalar" | "vector"
    relu_split=2,           # 0=all ACT, 1=all DVE, n>=2: fm%n==1 -> DVE
    h_copy="scalar",        # h1T/h2T copies
    at_mode="pe",           # attn prob transposes: "dma" | "pe"
)

_CACHE = {}


def _build(debug=False, repeat=1):
    nc = bacc.Bacc("TRN2", target_bir_lowering=False, debug=False,
                   num_devices=NCORES)

    # ---- DRAM I/O ----------------------------------------------------------
    x_d = nc.dram_tensor("x", [TOK, C], F32, kind="ExternalInput").ap()
    wq_d = nc.dram_tensor("wq", [C, C], F32R, kind="ExternalInput").ap()
    wk_d = nc.dram_tensor("wk", [C, C], F32R, kind="ExternalInput").ap()
    wv_d = nc.dram_tensor("wv", [C, C], F32R, kind="ExternalInput").ap()
    wp_d = nc.dram_tensor("wp", [C, C], F32R, kind="ExternalInput").ap()
    w1_d = nc.dram_tensor("w1", [C, FF], F32R, kind="ExternalInput").ap()
    w2_d = nc.dram_tensor("w2", [FF, C], F32R, kind="ExternalInput").ap()
    cqk_d = nc.dram_tensor("cqk", [128, 6], F32, kind="ExternalInput").ap()
    c1_d = nc.dram_tensor("c1", [128, 12], F32, kind="ExternalInput").ap()
    cvb_d = nc.dram_tensor("cvb", [128, C], F32, kind="ExternalInput").ap()
    bpb_d = nc.dram_tensor("bpb", [128, C], F32, kind="ExternalInput").ap()
    idr_d = nc.dram_tensor("idr", [128, 128], F32R, kind="ExternalInput").ap()
    idb_d = nc.dram_tensor("idb", [128, 128], BF16, kind="ExternalInput").ap()
    cmask_d = nc.dram_tensor("cmask", [128, 3 * 128], F32, kind="ExternalInput").ap()
    out_d = nc.dram_tensor("out", [TOK, C], F32, kind="ExternalOutput").ap()
    dbg = {}
    if debug:
        for nm, shape, dt in [
            ("d_h1", [128, NTT * C], F32R), ("d_h1T", [128, 3 * CTOK], F32R),
            ("d_qT", [128, 3 * CTOK], BF16), ("d_kT", [128, 3 * CTOK], BF16),
            ("d_v", [128, NTT * C], BF16), ("d_a0", [128, 128], BF16),
            ("d_a1", [128, 256], BF16), ("d_aT0", [128, 256], BF16),
            ("d_oT", [128, 3 * CTOK], F32R), ("d_x2", [128, NTT * C], F32),
            ("d_a1T", [128, 12 * CTOK], F32R), ("d_sums", [128, 2 * H], F32),
        ]:
            dbg[nm] = nc.dram_tensor(nm, shape, dt, kind="ExternalOutput").ap()

    with tile.TileContext(nc) as tc:
        with (
            tc.tile_pool(name="const", bufs=1) as cp,
            tc.tile_pool(name="io", bufs=2) as iop,
            tc.tile_pool(name="act", bufs=1) as ap_,
            tc.tile_pool(name="attn", bufs=13) as atp,
            tc.tile_pool(name="attnT", bufs=3) as aTp,
            tc.tile_pool(name="small", bufs=4) as smp,
            tc.tile_pool(name="mmps", bufs=2, space="PSUM") as mmp,
            tc.tile_pool(name="scps", bufs=3, space="PSUM") as scp,
            tc.tile_pool(name="tpps", bufs=2, space="PSUM") as tpp,
            tc.tile_pool(name="ops", bufs=1, space="PSUM") as opp,
        ):
            # ---- persistent weights / constants (issue order = first use) --
            idr_s = cp.tile([128, 128], F32R)
            nc.sync.dma_start(idr_s[:], idr_d[:])
            x_first = iop.tile([128, NTT * C], F32, tag="x")
            nc.sync.dma_start(
                x_first[:].rearrange("p (t c) -> p t c", t=NTT),
                x_d[0:CTOK, :].rearrange("(t p) c -> p t c", p=128))
            wq_s = cp.tile([128, 3 * C], F32R)
            wk_s = cp.tile([128, 3 * C], F32R)
            for c in range(3):
                nc.sync.dma_start(wq_s[:, C * c:C * (c + 1)], wq_d[128 * c:128 * (c + 1), :])
                nc.sync.dma_start(wk_s[:, C * c:C * (c + 1)], wk_d[128 * c:128 * (c + 1), :])
            cqk_s = cp.tile([128, 6], F32)
            nc.sync.dma_start(cqk_s[:], cqk_d[:])
            wv_s = cp.tile([128, 3 * C], F32R)
            for c in range(3):
                nc.sync.dma_start(wv_s[:, C * c:C * (c + 1)], wv_d[128 * c:128 * (c + 1), :])
            cvb_s = cp.tile([128, C], F32)
            nc.sync.dma_start(cvb_s[:], cvb_d[:])
            bpb_s = cp.tile([128, C], F32)
            nc.sync.dma_start(bpb_s[:], bpb_d[:])
            cmask_s = cp.tile([128, 3 * 128], F32)
            nc.sync.dma_start(cmask_s[:], cmask_d[:])
            idb_s = cp.tile([128, 128], BF16)
            nc.sync.dma_start(idb_s[:], idb_d[:])
            wp_s = cp.tile([128, 3 * C], F32R)
            for c in range(3):
                nc.sync.dma_start(wp_s[:, C * c:C * (c + 1)], wp_d[128 * c:128 * (c + 1), :])
            w1_s = cp.tile([128, 3 * FF], F32R)
            for c in range(3):
                nc.sync.dma_start(w1_s[:, FF * c:FF * (c + 1)], w1_d[128 * c:128 * (c + 1), :])
            c1_s = cp.tile([128, 12], F32)
            nc.sync.dma_start(c1_s[:], c1_d[:])
            w2_s = cp.tile([128, 12 * C], F32R)
            for f in range(12):
                nc.sync.dma_start(w2_s[:, C * f:C * (f + 1)], w2_d[128 * f:128 * (f + 1), :])

            def load_x(ch):
                base = (ch % NCH) * CTOK
                t = iop.tile([128, NTT * C], F32, tag="x")
                nc.sync.dma_start(
                    t[:].rearrange("p (t c) -> p t c", t=NTT),
                    x_d[base:base + CTOK, :].rearrange("(t p) c -> p t c", p=128))
                return t

            x_next = x_first
            for ch in range(NCH * repeat):
                ch_next = ch + 1
                ch = ch % NCH
                base = ch * CTOK

                x_sb = x_next
                if ch_next < NCH * repeat:
                    x_next = load_x(ch_next)

                # ---- LN1 (token-major) -> h1_z (fp32r) ---------------------
                s1 = smp.tile([128, NTT], F32, tag="s1")
                s2 = smp.tile([128, NTT], F32, tag="s2")
                sq = ap_.tile([128, C], F32, tag="sq")
                for tt in range(NTT):
                    xt = x_sb[:, C * tt:C * (tt + 1)]
                    getattr(nc, CFG["s1_engine"]).reduce_sum(
                        s1[:, tt:tt + 1], xt, axis=mybir.AxisListType.X)
                    nc.scalar.activation(sq[:], xt, AF.Square, bias=0.0, scale=1.0,
                                         accum_out=s2[:, tt:tt + 1])
                mu = smp.tile([128, NTT], F32, tag="mu")
                nc.vector.tensor_scalar_mul(mu[:], s1[:], 1.0 / C)
                var = smp.tile([128, NTT], F32, tag="var")
                nc.vector.tensor_scalar(var[:], s2[:], 1.0 / C, EPS,
                                        op0=ALU.mult, op1=ALU.add)
                musq = smp.tile([128, NTT], F32, tag="musq")
                nc.vector.tensor_tensor(musq[:], mu[:], mu[:], op=ALU.mult)
                nc.vector.tensor_tensor(var[:], var[:], musq[:], op=ALU.subtract)
                sd = smp.tile([128, NTT], F32, tag="sd")
                nc.scalar.activation(sd[:], var[:], AF.Sqrt, bias=0.0, scale=1.0)
                rstd = smp.tile([128, NTT], F32, tag="rstd")
                nc.vector.reciprocal(rstd[:], sd[:])

                h1 = ap_.tile([128, NTT * C], F32R, tag="h1")
                for tt in range(NTT):
                    nc.vector.tensor_scalar(
                        h1[:, C * tt:C * (tt + 1)], x_sb[:, C * tt:C * (tt + 1)],
                        mu[:, tt:tt + 1], rstd[:, tt:tt + 1],
                        op0=ALU.subtract, op1=ALU.mult)

                # ---- transpose h1 -> h1T [c-tile partitions, tokens] -------
                h1T = ap_.tile([128, 3 * CTOK], F32R, tag="h1T")
                for c in range(3):
                    tp = scp.tile([128, CTOK], F32R, tag="sc")
                    for tt in range(NTT):
                        nc.tensor.matmul(
                            tp[:, 128 * tt:128 * (tt + 1)],
                            h1[:, C * tt + 128 * c:C * tt + 128 * (c + 1)], idr_s[:],
                            is_transpose=True, start=(tt == 0), stop=(tt == NTT - 1))
                    if CFG["h_copy"] == "scalar":
                        nc.scalar.copy(h1T[:, CTOK * c:CTOK * (c + 1)], tp[:])
                    else:
                        nc.vector.tensor_copy(h1T[:, CTOK * c:CTOK * (c + 1)], tp[:])

                if debug and ch == 0:
                    nc.sync.dma_start(dbg["d_h1"][:], h1[:])
                    nc.sync.dma_start(dbg["d_h1T"][:], h1T[:])

                # ---- Q^T, K^T (feature-major, bf16, bias folded) -----------
                qT = ap_.tile([128, 3 * CTOK], BF16, tag="qT")
                kT = ap_.tile([128, 3 * CTOK], BF16, tag="kT")
                for w_s, oT, bcol in ((wq_s, qT, 0), (wk_s, kT, 3)):
                    for m in range(3):
                        ps = mmp.tile([128, CTOK], F32, tag="mm")
                        for c in range(3):
                            nc.tensor.matmul(
                                ps[:],
                                w_s[:, C * c + 128 * m:C * c + 128 * (m + 1)],
                                h1T[:, CTOK * c:CTOK * (c + 1)],
                                start=(c == 0), stop=(c == 2))
                        if CFG["qk_copy"] == "scalar":
                            nc.scalar.activation(
                                oT[:, CTOK * m:CTOK * (m + 1)], ps[:], AF.Identity,
                                bias=cqk_s[:, bcol + m:bcol + m + 1], scale=1.0)
                        else:
                            nc.vector.tensor_scalar_add(
                                oT[:, CTOK * m:CTOK * (m + 1)], ps[:],
                                cqk_s[:, bcol + m:bcol + m + 1])

                # ---- V (token-major, bf16, bias folded) --------------------
                v_sb = ap_.tile([128, NTT * C], BF16, tag="v")
                for tt in range(NTT):
                    ps = mmp.tile([128, C], F32, tag="mm")
                    for c in range(3):
                        nc.tensor.matmul(
                            ps[:],
                            h1T[:, CTOK * c + 128 * tt:CTOK * c + 128 * (tt + 1)],
                            wv_s[:, C * c:C * (c + 1)],
                            start=(c == 0), stop=(c == 2))
                    nc.vector.tensor_tensor(
                        v_sb[:, C * tt:C * (tt + 1)], ps[:], cvb_s[:], op=ALU.add)

                if debug and ch == 0:
                    nc.sync.dma_start(dbg["d_qT"][:], qT[:])
                    nc.sync.dma_start(dbg["d_kT"][:], kT[:])
                    nc.sync.dma_start(dbg["d_v"][:], v_sb[:])

                # ---- attention, per batch in chunk -------------------------
                oT = ap_.tile([128, 3 * CTOK], F32R, tag="oT")
                x2 = ap_.tile([128, NTT * C], F32, tag="x2")
                for bb in range(CHB):
                    t0, t1 = 2 * bb, 2 * bb + 1     # token tiles of this batch
                    sums = smp.tile([128, 2 * H], F32, tag="sums")
                    attn0 = []
                    attn1 = []
                    for h in range(H):
                        qp = 64 * (h % 2)
                        qm = h // 2
                        q_t0 = qT[qp:qp + 64, CTOK * qm + 128 * t0:CTOK * qm + 128 * (t0 + 1)]
                        q_t1 = qT[qp:qp + 64, CTOK * qm + 128 * t1:CTOK * qm + 128 * (t1 + 1)]
                        k_s0 = kT[qp:qp + 64, CTOK * qm + 128 * t0:CTOK * qm + 128 * (t0 + 1)]
                        k_s01 = kT[qp:qp + 64, CTOK * qm + 128 * t0:CTOK * qm + 128 * (t0 + 2)]

                        sc = scp.tile([128, 384], F32, tag="sc")
                        nc.tensor.matmul(sc[:, 0:128], q_t0, k_s0, start=True, stop=False)
                        nc.tensor.matmul(sc[:, 128:384], q_t1, k_s01, start=False, stop=True)
                        nc.vector.tensor_tensor(sc[:], sc[:], cmask_s[:], op=ALU.add)
                        a0 = atp.tile([128, 128], BF16, tag="a0")
                        nc.scalar.activation(a0[:], sc[:, 0:128], AF.Exp, bias=0.0, scale=1.0,
                                             accum_out=sums[:, 2 * h:2 * h + 1])
                        attn0.append(a0)
                        a1 = atp.tile([128, 256], BF16, tag="a1")
                        nc.scalar.activation(a1[:], sc[:, 128:384], AF.Exp, bias=0.0, scale=1.0,
                                             accum_out=sums[:, 2 * h + 1:2 * h + 2])
                        attn1.append(a1)

                    rec = smp.tile([128, 2 * H], F32, tag="rec")
                    nc.vector.reciprocal(rec[:], sums[:])

                    for h in range(H):
                        a0, a1 = attn0[h], attn1[h]
                        nc.vector.tensor_scalar_mul(a0[:], a0[:], rec[:, 2 * h:2 * h + 1])
                        nc.vector.tensor_scalar_mul(a1[:], a1[:], rec[:, 2 * h + 1:2 * h + 2])

                        # transpose probs:  aT_s0 [s0, 256 t], aT_s1 [s1, t1 only]
                        aT0 = aTp.tile([128, 256], BF16, tag="aT0")
                        aT1 = aTp.tile([128, 128], BF16, tag="aT1")
                        if CFG["at_mode"] == "dma":
                            nc.sync.dma_start_transpose(aT0[:, 0:128], a0[:])
                            nc.sync.dma_start_transpose(aT0[:, 128:256], a1[:, 0:128])
                            nc.sync.dma_start_transpose(aT1[:], a1[:, 128:256])
                        else:
                            tp0 = tpp.tile([128, 256], BF16, tag="tpa")
                            nc.tensor.matmul(tp0[:, 0:128], a0[:], idb_s[:],
                                             is_transpose=True, start=True, stop=False)
                            nc.tensor.matmul(tp0[:, 128:256], a1[:, 0:128], idb_s[:],
                                             is_transpose=True, start=False, stop=True)
                            nc.vector.tensor_copy(aT0[:], tp0[:])
                            tp2 = tpp.tile([128, 256], BF16, tag="tpa")
                            nc.tensor.matmul(tp2[:, 0:128], a1[:, 128:256], idb_s[:],
                                             is_transpose=True, start=True, stop=True)
                            nc.vector.tensor_copy(aT1[:], tp2[:, 0:128])

                        if debug and ch == 0 and bb == 0 and h == 0:
                            nc.sync.dma_start(dbg["d_a0"][:], a0[:])
                            nc.sync.dma_start(dbg["d_a1"][:], a1[:])
                            nc.sync.dma_start(dbg["d_aT0"][:], aT0[:])

                        # attn @ V -> OT psum [64 d, 256 t] per head
                        ot_ps = opp.tile([64, 256], F32, tag="ot")
                        nc.tensor.matmul(
                            ot_ps[:],
                            v_sb[:, C * t0 + 64 * h:C * t0 + 64 * (h + 1)],
                            aT0[:], start=True, stop=False)
                        nc.tensor.matmul(
                            ot_ps[:, 128:256],
                            v_sb[:, C * t1 + 64 * h:C * t1 + 64 * (h + 1)],
                            aT1[:], start=False, stop=True)
                        hp, op = h // 2, 64 * (h % 2)
                        dst = oT[op:op + 64, CTOK * hp + 256 * bb:CTOK * hp + 256 * (bb + 1)]
                        if CFG["ot_copy"] == "scalar":
                            nc.scalar.copy(dst, ot_ps[:])
                        else:
                            nc.vector.tensor_copy(dst, ot_ps[:])

                if debug and ch == 0:
                    nc.sync.dma_start(dbg["d_oT"][:], oT[:])
                    nc.sync.dma_start(dbg["d_sums"][:], sums[:])


                if debug and ch == 0:
                    nc.sync.dma_start(dbg["d_x2"][:], x2[:])

                # ---- proj + residual + bp -> x2 ----------------------------
                for tt in range(NTT):
                    ps = mmp.tile([128, C], F32, tag="mm")
                    for c in range(3):
                        nc.tensor.matmul(
                            ps[:],
                            oT[:, CTOK * c + 128 * tt:CTOK * c + 128 * (tt + 1)],
                            wp_s[:, C * c:C * (c + 1)],
                            start=(c == 0), stop=(c == 2))
                    nc.vector.tensor_tensor(
                        ps[:], ps[:], x_sb[:, C * tt:C * (tt + 1)], op=ALU.add)
                    nc.vector.tensor_tensor(
                        x2[:, C * tt:C * (tt + 1)], ps[:], bpb_s[:], op=ALU.add)

                # ---- LN2 -> h2_z (fp32r) -----------------------------------
                for tt in range(NTT):
                    xt = x2[:, C * tt:C * (tt + 1)]
                    getattr(nc, CFG["s1_engine"]).reduce_sum(
                        s1[:, tt:tt + 1], xt, axis=mybir.AxisListType.X)
                    nc.scalar.activation(sq[:], xt, AF.Square, bias=0.0, scale=1.0,
                                         accum_out=s2[:, tt:tt + 1])
                nc.vector.tensor_scalar_mul(mu[:], s1[:], 1.0 / C)
                nc.vector.tensor_scalar(var[:], s2[:], 1.0 / C, EPS,
                                        op0=ALU.mult, op1=ALU.add)
                nc.vector.tensor_tensor(musq[:], mu[:], mu[:], op=ALU.mult)
                nc.vector.tensor_tensor(var[:], var[:], musq[:], op=ALU.subtract)
                nc.scalar.activation(sd[:], var[:], AF.Sqrt, bias=0.0, scale=1.0)
                nc.vector.reciprocal(rstd[:], sd[:])
                h2 = ap_.tile([128, NTT * C], F32R, tag="h2")
                for tt in range(NTT):
                    nc.vector.tensor_scalar(
                        h2[:, C * tt:C * (tt + 1)], x2[:, C * tt:C * (tt + 1)],
                        mu[:, tt:tt + 1], rstd[:, tt:tt + 1],
                        op0=ALU.subtract, op1=ALU.mult)

                h2T = ap_.tile([128, 3 * CTOK], F32R, tag="h2T")
                for c in range(3):
                    tp = scp.tile([128, CTOK], F32R, tag="sc")
                    for tt in range(NTT):
                        nc.tensor.matmul(
                            tp[:, 128 * tt:128 * (tt + 1)],
                            h2[:, C * tt + 128 * c:C * tt + 128 * (c + 1)], idr_s[:],
                            is_transpose=True, start=(tt == 0), stop=(tt == NTT - 1))
                    if CFG["h_copy"] == "scalar":
                        nc.scalar.copy(h2T[:, CTOK * c:CTOK * (c + 1)], tp[:])
                    else:
                        nc.vector.tensor_copy(h2T[:, CTOK * c:CTOK * (c + 1)], tp[:])

                # ---- MLP1: a1T = relu(w1'^T h2T + c1)  (fp32r) -------------
                a1T = ap_.tile([128, 12 * CTOK], F32R, tag="a1T")
                for fm in range(12):
                    ps = mmp.tile([128, CTOK], F32, tag="mm")
                    for c in range(3):
                        nc.tensor.matmul(
                            ps[:],
                            w1_s[:, FF * c + 128 * fm:FF * c + 128 * (fm + 1)],
                            h2T[:, CTOK * c:CTOK * (c + 1)],
                            start=(c == 0), stop=(c == 2))
                    if CFG["relu_split"] == 1 or (
                            CFG["relu_split"] > 1 and fm % CFG["relu_split"] == 1):
                        nc.vector.tensor_scalar(
                            a1T[:, CTOK * fm:CTOK * (fm + 1)], ps[:],
                            c1_s[:, fm:fm + 1], 0.0, op0=ALU.add, op1=ALU.max)
                    else:
                        nc.scalar.activation(
                            a1T[:, CTOK * fm:CTOK * (fm + 1)], ps[:], AF.Relu,
                            bias=c1_s[:, fm:fm + 1], scale=1.0)

                if debug and ch == 0:
                    nc.sync.dma_start(dbg["d_a1T"][:], a1T[:])

                # ---- MLP2 + residual -> out --------------------------------
                o_sb = iop.tile([128, NTT * C], F32, tag="o")
                for tt in range(NTT):
                    ps = mmp.tile([128, C], F32, tag="mm")
                    for fm in range(12):
                        nc.tensor.matmul(
                            ps[:],
                            a1T[:, CTOK * fm + 128 * tt:CTOK * fm + 128 * (tt + 1)],
                            w2_s[:, C * fm:C * (fm + 1)],
                            start=(fm == 0), stop=(fm == 11))
                    nc.vector.tensor_tensor(
                        o_sb[:, C * tt:C * (tt + 1)], ps[:],
                        x2[:, C * tt:C * (tt + 1)], op=ALU.add)
                nc.sync.dma_start(
                    out_d[base:base + CTOK, :].rearrange("(t p) c -> p t c", p=128),
                    o_sb[:].rearrange("p (t c) -> p t c", t=NTT))

    nc.compile()
    return nc


def _prep_inputs(x, wq, wk, wv, wp, bp, w1, w2, g1, b1, g2, b2):
    """Host-side weight folding + per-core input maps."""
    f32 = np.float32
    scale = HD ** -0.5
    wq_m = np.ascontiguousarray(wq.transpose(1, 0, 2).reshape(C, C))
    wk_m = np.ascontiguousarray(wk.transpose(1, 0, 2).reshape(C, C))
    wv_m = np.ascontiguousarray(wv.transpose(1, 0, 2).reshape(C, C))
    wq_p = (g1[:, None] * wq_m * scale).astype(f32)
    cq = (b1 @ wq_m * scale).astype(f32)
    wk_p = (g1[:, None] * wk_m).astype(f32)
    ck = (b1 @ wk_m).astype(f32)
    wv_p = (g1[:, None] * wv_m).astype(f32)
    cv = (b1 @ wv_m).astype(f32)
    w1_p = (g2[:, None] * w1).astype(f32)
    c1 = (b2 @ w1).astype(f32)

    cqk = np.concatenate([cq.reshape(3, 128).T, ck.reshape(3, 128).T], axis=1)
    cqk = np.ascontiguousarray(cqk, dtype=f32)                       # [128, 6]
    c1t = np.ascontiguousarray(c1.reshape(12, 128).T, dtype=f32)     # [128, 12]
    cvb = np.ascontiguousarray(np.broadcast_to(cv, (128, C)), dtype=f32)
    bpb = np.ascontiguousarray(np.broadcast_to(np.asarray(bp, f32), (128, C)))
    idr = np.eye(128, dtype=f32)
    idb = np.eye(128).astype(ml_dtypes.bfloat16)
    ii, jj = np.arange(128)[:, None], np.arange(128)[None, :]
    tri = np.where(jj <= ii, 0.0, NEG).astype(f32)
    cmask = np.concatenate([tri, np.zeros((128, 128), f32), tri], axis=1)

    common = dict(wq=wq_p, wk=wk_p, wv=wv_p, wp=np.asarray(wp, f32),
                  w1=w1_p, w2=np.asarray(w2, f32), cqk=cqk, c1=c1t,
                  cvb=cvb, bpb=bpb, idr=idr, idb=idb, cmask=cmask)
    in_maps = []
    for core in range(NCORES):
        xs = np.ascontiguousarray(
            x[BL * core:BL * (core + 1)].reshape(TOK, C), dtype=f32)
        in_maps.append(dict(common, x=xs))
    return in_maps


def run(inputs, trace=False, trace_kwargs=None, debug=False):
    """Compile (cached), run on 8 cores, gather. Returns (out, results)."""
    key = "nc_dbg" if debug else "nc"
    if key not in _CACHE:
        _CACHE[key] = _build(debug=debug)
    nc = _CACHE[key]
    in_maps = _prep_inputs(**inputs)
    res = run_bass_kernel_spmd(nc, in_maps, list(range(NCORES)),
                               trace=trace, **(trace_kwargs or {}))
    out = np.empty((B, T, C), dtype=np.float32)
    for core in range(NCORES):
        out[BL * core:BL * (core + 1)] = \
            res.results[core]["out"].reshape(BL, T, C)
    return out, res


def kernel(**inputs):
    out, _ = run(inputs)
    return out
